# revision 5
# baseline (speedup 1.0000x reference)
"""Segment-mean pooling (AvgPoolingLayer / segment_reduce) on 8 Trainium2 cores.

Strategy
--------
segment_ids are sorted, so each segment occupies a contiguous row range.
Shard rows across 8 cores at segment boundaries (each segment lives on
exactly one core).  Per core, the segment-sum is computed as a chain of
one-hot matmuls on the PE:

    psum[block] += one_hot(ids_tile)^T @ feats_tile

where one_hot is built from a precomputed "relative id" input compared
against an iota constant.  One-hot builds alternate between the DVE and
the GpSimd (Pool) engine so neither becomes the bottleneck.

Precision: feats are converted to a single bf16 copy on the host
(2 B/element — max rel err of the segment means ~2e-3, well under the
2e-2 gate), halving HBM traffic vs an fp32 or bf16-hi/lo scheme.  The
PE consumes bf16 at 1 cycle/row and accumulates fp32 in PSUM.

Block schedule: PSUM blocks cover 128 segments but START every 64
segments (stride-64 overlap).  Each 128-row matmul tile is assigned,
statically and identically for all cores, to the one block that
contains every core's segment span for that tile (possible because the
per-tile span ~20 segs + cross-core skew ~±12 segs < 64).  This
removes the baseline's ~22% duplicated matmuls/one-hots from tiles
straddling a block boundary.  The 64-segment overlap between
consecutive blocks is resolved by one constant-weight shift matmul per
block: rows [64:128) of block k (copied PSUM->SBUF in bf16 on the
otherwise idle Activation engine) are added into rows [0:64) of block
k+1 via an identity-like [128,64] weight.  Block k then outputs rows
[0:64) only (the last block outputs all 128).

DMA layout: rows are assigned to SBUF partitions chunk-wise
(partition p of a 2048-row chunk holds rows [16p, 16p+16)), which makes
every feats DMA a fully linear HBM read with 8 KiB contiguous packets
per partition.  The row permutation is absorbed into the precomputed
rel inputs.

SPMD: one Bass program runs on all 8 cores; all per-core differences
(row windows, relative ids, inverse counts) are carried in the input
data, never in the instruction stream.
"""

import numpy as np
import ml_dtypes

from concourse import bass, mybir, tile
from concourse.bass_utils import run_bass_kernel_spmd

N = 1_000_000
D = 256
S = 10_000
NCORES = 8
P = 128           # rows per matmul tile == SBUF partitions
CHUNK = 16        # tiles per feats DMA == consecutive rows per partition
SPC = S // NCORES # segments owned per core
BSTRIDE = 64      # segment stride between (overlapping) PSUM blocks
NBLK = (SPC - P + BSTRIDE - 1) // BSTRIDE + 1  # blocks cover [0, 64k+128)

_f32 = mybir.dt.float32
_bf16 = mybir.dt.bfloat16


def _plan(ids, n_rows, n_cores, segs_per_core, nblk, chunk):
    """Host-side plan: per-core row windows + static (tile, block) issue list.

    Row order is partition-major within each P*chunk-row chunk: tile
    (c, n) covers rows {chunk_start + chunk*p + n : p in 0..P-1}.
    Blocks overlap: block b covers local segments [64b, 64b+128).
    Every tile gets one slot per block needed so that EVERY core's rows
    are covered; each row is assigned (via rel) to the first covering
    slot of its tile.  Returns (starts, R, issue, rel, first_slot,
    last_slot).
    """
    g = np.arange(n_cores + 1, dtype=np.int64) * segs_per_core
    b_rows = np.searchsorted(ids, g, side="left")
    spans = b_rows[1:] - b_rows[:-1]
    R = int(np.ceil(spans.max() / (P * chunk)) * (P * chunk))
    assert R <= n_rows and R >= spans.max()
    starts = np.minimum(b_rows[:-1], n_rows - R)
    T = R // P
    nchunk = T // chunk

    # per-core local segment index of every row in its window, reshaped
    # to the partition-major tile order: [C, nchunk, P, chunk]
    vals = np.stack([ids[s:s + R] for s in starts]).astype(np.int64)
    vals -= g[:-1, None]
    vals_t = vals.reshape(n_cores, nchunk, P, chunk)
    owned = (vals_t >= 0) & (vals_t < segs_per_core)

    issue = []
    tile_blocks = []
    for c in range(nchunk):
        for n in range(chunk):
            v = vals_t[:, c, :, n]
            ok = owned[:, c, :, n]
            if not ok.any():
                tile_blocks.append([])
                continue
            lo = int(v[ok].min())
            hi = int(v[ok].max())
            b = min(lo // BSTRIDE, nblk - 1)
            bs = [b]
            while hi >= BSTRIDE * bs[-1] + P:
                nb = min(bs[-1] + 2, hi // BSTRIDE, nblk - 1)
                assert nb > bs[-1], (lo, hi, bs)
                bs.append(nb)
            assert hi < BSTRIDE * bs[-1] + P
            t = c * chunk + n
            issue.extend((t, b) for b in bs)
            tile_blocks.append(bs)

    n_slots = len(issue)
    rel = np.full((n_cores, P, n_slots), -1.0, dtype=np.float32)
    assigned = np.zeros((n_cores, nchunk, P, chunk), dtype=bool)
    for i, (t, b) in enumerate(issue):
        v = vals_t[:, t // chunk, :, t % chunk]
        ok = owned[:, t // chunk, :, t % chunk]
        a = assigned[:, t // chunk, :, t % chunk]
        w = v - b * BSTRIDE
        hit = ok & ~a & (w >= 0) & (w < P)
        rel[:, :, i] = np.where(hit, w, -1).astype(np.float32)
        a |= hit
    assert assigned[owned].all(), "some rows not covered by any slot"

    first_slot, last_slot = {}, {}
    for i, (t, b) in enumerate(issue):
        first_slot.setdefault(b, i)
        last_slot[b] = i
    assert set(first_slot) == set(range(nblk)), (
        f"blocks missing from issue list: {sorted(set(range(nblk)) - set(first_slot))}"
    )
    # the shift chain requires block b to stop strictly after block b-1
    for b in range(1, nblk):
        assert last_slot[b] > last_slot[b - 1], (b, last_slot)
    return starts, R, issue, rel, first_slot, last_slot


def _build_program(R, d, nblk, issue, first_slot, last_slot, chunk):
    """Emit the SPMD Bass program (identical for all cores)."""
    T = R // P
    n_slots = len(issue)
    out_rows = BSTRIDE * (nblk - 1) + P
    nc = bass.Bass()
    hb_d = nc.dram_tensor("hb", [R, d], _bf16, kind="ExternalInput")
    # iota is bf16 (fast DVE input); rel must be f32 (tensor_scalar
    # scalar operand), packed with inv + shift-scalar so one DMA covers all
    iota_d = nc.dram_tensor("iota", [P, P], _bf16, kind="ExternalInput")
    meta_d = nc.dram_tensor("meta", [P, n_slots + nblk + 1], _f32,
                            kind="ExternalInput")
    out_d = nc.dram_tensor("out", [out_rows, d], _f32, kind="ExternalOutput")

    with tile.TileContext(nc) as tc:
        with (
            tc.tile_pool(name="const", bufs=1) as cpool,
            tc.tile_pool(name="feats", bufs=6) as fpool,
            tc.tile_pool(name="oh", bufs=8) as ohpool,
            tc.tile_pool(name="acc", bufs=5, space=bass.MemorySpace.PSUM) as pspool,
            tc.tile_pool(name="cpy", bufs=3) as cppool,
            tc.tile_pool(name="res", bufs=3) as rpool,
        ):
            iota_tile = cpool.tile([P, P], _bf16)
            nc.sync.dma_start(iota_tile[:], iota_d[:])
            meta_t = cpool.tile([P, n_slots + nblk + 1], _f32)
            nc.sync.dma_start(meta_t[:], meta_d[:])
            iota_t = iota_tile[:]
            rel_t = meta_t[:, 0:n_slots]
            inv_t = meta_t[:, n_slots:n_slots + nblk]
            shsc_t = meta_t[:, n_slots + nblk:]

            # shift weights: ones at (64+m, m) — built once on the DVE
            shift_w = cpool.tile([P, BSTRIDE], _bf16, name="shift_w")
            nc.vector.tensor_scalar(
                out=shift_w[:], in0=iota_t[:, 0:BSTRIDE],
                scalar1=shsc_t, scalar2=None,
                op0=mybir.AluOpType.is_equal)

            # PE warm-up: dummy matmuls while the first feats chunk is in
            # flight keep the PE busy so the clock ramps (0.65/1.2 ->
            # 2.4 GHz) before real work arrives.
            warm = cpool.tile([P, P], _bf16, name="warm")
            nc.vector.memset(warm[:], 0.0)
            warm_rhs = cpool.tile([P, d], _bf16, name="warm_rhs")
            nc.vector.memset(warm_rhs[:], 0.0)
            wacc = pspool.tile([P, d], _f32, name="wacc", tag="acc")
            for _ in range(24):
                nc.tensor.matmul(wacc[:], warm[:], warm_rhs[:],
                                 start=True, stop=True)

            psum_tiles = {}
            started = set()
            pending_shifts = []  # (emit_at_slot, b, pt) — deferred copy+shift

            # Defer the PSUM->SBUF copy + shift matmul a few slots past
            # block b's stop so the PE doesn't stall waiting on the
            # Activation copy at each block transition.
            SHIFT_DELAY = 6

            def emit_shift(b, pt):
                # shift block b's top 64 rows into block b+1's bottom 64
                # (constant-weight matmul via a bf16 SBUF copy made on the
                # otherwise idle Activation engine).
                cp = cppool.tile([P, d], _bf16, name="cpy", tag="cpy")
                nc.scalar.activation(
                    cp[:], pt[:, :], mybir.ActivationFunctionType.Copy)
                if b + 1 not in psum_tiles:
                    psum_tiles[b + 1] = pspool.tile(
                        [P, d], _f32, name="acc", tag="acc")
                nc.tensor.matmul(psum_tiles[b + 1][0:BSTRIDE, :],
                                 shift_w[:], cp[:],
                                 start=(b + 1) not in started,
                                 stop=False, skip_group_check=True)
                started.add(b + 1)

            def emit_scale(b, pt):
                # block b is fully accumulated: scale by 1/count, DMA out.
                # scale must run on the DVE: GpSimd cannot access PSUM
                rows = P if b == nblk - 1 else BSTRIDE
                res = rpool.tile([rows, d], _f32, name="res", tag="res")
                nc.vector.tensor_scalar(
                    out=res[:], in0=pt[0:rows, :],
                    scalar1=inv_t[0:rows, b:b + 1], scalar2=None,
                    op0=mybir.AluOpType.mult)
                nc.scalar.dma_start(
                    out_d[b * BSTRIDE:b * BSTRIDE + rows, :], res[:])

            slot = 0
            for c in range(T // chunk):
                hl = fpool.tile([P, chunk, d], _bf16)
                r0 = c * chunk * P
                src = hb_d[r0:r0 + chunk * P].rearrange(
                    "(p n) d -> p n d", p=P)
                nc.sync.dma_start(hl[:], src)
                for j in range(chunk):
                    t = c * chunk + j
                    while slot < n_slots and issue[slot][0] == t:
                        while pending_shifts and pending_shifts[0][0] <= slot:
                            _, pb, ppt = pending_shifts.pop(0)
                            emit_shift(pb, ppt)
                        b = issue[slot][1]
                        oh = ohpool.tile([P, P], _bf16)
                        eng = nc.vector if slot % 2 == 0 else nc.gpsimd
                        eng.tensor_scalar(
                            out=oh[:], in0=iota_t,
                            scalar1=rel_t[:, slot:slot + 1], scalar2=None,
                            op0=mybir.AluOpType.is_equal)
                        if b not in psum_tiles:
                            assert slot == first_slot[b], (slot, b)
                            psum_tiles[b] = pspool.tile(
                                [P, d], _f32, name="acc", tag="acc")
                        pt = psum_tiles[b]
                        nc.tensor.matmul(pt[:, :], oh[:], hl[:, j, :],
                                         start=(slot == first_slot[b]
                                                and b not in started),
                                         stop=(slot == last_slot[b]),
                                         skip_group_check=True)
                        started.add(b)
                        if slot == last_slot[b]:
                            pt = psum_tiles.pop(b)
                            if b + 1 < nblk:
                                delay = min(SHIFT_DELAY,
                                            last_slot[b + 1] - slot - 1)
                                pending_shifts.append((slot + delay, b, pt))
                            emit_scale(b, pt)
                        slot += 1
            for _, pb, ppt in pending_shifts:
                emit_shift(pb, ppt)
    assert slot == n_slots
    _strip_self_waits(nc)
    _legalize_waits(nc)
    return nc


# Compute ops whose ISA structs carry a single sync-wait slot.  Tile's
# pool-slot release join sometimes adds a same-engine WAW/WAR wait on top
# of a cross-engine one; same-engine ordering is already guaranteed by
# in-order execution (Tile records same-engine deps as no-sync edges
# elsewhere), so the self-wait is redundant and safe to drop.
_COMPUTE_OPS = (
    mybir.InstTensorTensor, mybir.InstTensorScalarPtr,
    mybir.InstTensorCopy, mybir.InstActivation, mybir.InstMemset,
    mybir.InstMatmult, mybir.InstLdweights, mybir.InstTensorReduce,
)

_COMPUTE_SEMS = ("PE_", "DVE_", "Pool_", "Activation_", "SP_")


def _strip_self_waits(nc):
    for bb in nc.main_func.blocks:
        for ins in bb.instructions:
            si = ins.sync_info
            if si is None or not si.on_wait:
                continue
            if isinstance(ins, _COMPUTE_OPS):
                eng = str(ins.engine).split(".")[-1]
                kept = [w for w in si.on_wait
                        if not w.ant_name.startswith(eng + "_")]
                if len(kept) != len(si.on_wait):
                    si.on_wait = kept
            elif isinstance(ins, mybir.InstDMACopy) and len(si.on_wait) > 1:
                # A WAW wait on the old writer's DMA queue is implied by the
                # compute-engine wait that gates on the old tile's readers
                # (the readers FIFO-follow a wait on that very queue).
                has_compute = any(
                    w.ant_name.startswith(_COMPUTE_SEMS) for w in si.on_wait)
                if has_compute:
                    kept = [w for w in si.on_wait
                            if not w.ant_name.startswith("DMAHW")]
                    if kept and len(kept) != len(si.on_wait):
                        si.on_wait = kept


def _legalize_waits(nc, maxw=1):
    """The walrus codegen here supports very few sync-wait commands per
    instruction.  Hoist excess waits onto preceding same-engine NoOps —
    engine FIFO order makes this equivalent."""
    for bb in nc.main_func.blocks:
        idx = 0
        while idx < len(bb.instructions):
            ins = bb.instructions[idx]
            si = ins.sync_info
            if si is not None and si.on_wait and len(si.on_wait) > maxw:
                waits = list(si.on_wait)
                si.on_wait = waits[-maxw:]
                for w in waits[:-maxw]:
                    nop = mybir.InstNoOp(
                        name=nc.get_next_instruction_name(),
                        engine=ins.engine,
                        sync_info=mybir.SyncInfo(on_wait=[w], on_update=[]),
                        bass_nofuse=True,
                    )
                    bb.instructions.insert(idx, nop)
                    idx += 1
            idx += 1


def _prepare_inputs(feats, ids, n_cores, segs_per_core, nblk, starts, R, rel):
    """Per-core input maps: bf16 feats + meta (rel | inv | shift scalar)."""
    n, d = feats.shape
    counts = np.bincount(ids, minlength=n_cores * segs_per_core).astype(np.float32)
    inv = (1.0 / np.maximum(counts, 1.0)).astype(np.float32)
    inv_pad = np.zeros(n_cores * segs_per_core + nblk * BSTRIDE + P, np.float32)
    inv_pad[:inv.shape[0]] = inv

    hb = feats.astype(ml_dtypes.bfloat16)

    n_slots = rel.shape[2]
    # iota[p, j] = j — compared against rel[p] to build the one-hot
    iota = np.broadcast_to(np.arange(P, dtype=np.float32), (P, P))
    shsc = (np.arange(P, dtype=np.float32) - BSTRIDE).reshape(P, 1)
    in_maps = []
    for c in range(n_cores):
        g0 = c * segs_per_core
        inv_c = inv_pad[g0:g0 + nblk * BSTRIDE + P].copy()
        inv_c[segs_per_core:] = 0.0
        meta = np.empty((P, n_slots + nblk + 1), np.float32)
        meta[:, 0:n_slots] = rel[c]
        for b in range(nblk):
            meta[:, n_slots + b] = inv_c[b * BSTRIDE:b * BSTRIDE + P]
        meta[:, n_slots + nblk:] = shsc
        in_maps.append({
            "hb": hb[starts[c]:starts[c] + R],
            "iota": iota.astype(ml_dtypes.bfloat16),
            "meta": meta,
        })
    return in_maps


def _run(feats, ids, trace=False, trace_cores=None):
    n, d = feats.shape
    starts, R, issue, rel, first_slot, last_slot = _plan(
        ids, n, NCORES, SPC, NBLK, CHUNK)
    nc = _build_program(R, d, NBLK, issue, first_slot, last_slot, CHUNK)
    in_maps = _prepare_inputs(feats, ids, NCORES, SPC, NBLK,
                              starts, R, rel)
    res = run_bass_kernel_spmd(nc, in_maps, list(range(NCORES)),
                               trace=trace, trace_cores=trace_cores)
    out = np.concatenate(
        [res.results[c]["out"][:SPC] for c in range(NCORES)], axis=0)
    return out, res


def kernel(feats, segment_ids, num_segments):
    feats = np.ascontiguousarray(np.asarray(feats), dtype=np.float32)
    ids = np.asarray(segment_ids).astype(np.int64)
    s = int(num_segments)
    assert feats.shape == (N, D) and ids.shape == (N,) and s == S, (
        "kernel is specialized for feats [1e6, 256], 1e4 segments")
    out, _ = _run(feats, ids)
    return out


# revision 8
# speedup vs baseline: 4.7849x; 4.7849x over previous
"""Segment-mean pooling (AvgPoolingLayer / segment_reduce) on 8 Trainium2 cores.

Strategy
--------
segment_ids are sorted, so each segment occupies a contiguous row range.
Shard rows across 8 cores at segment boundaries (each segment lives on
exactly one core).  Per core, the segment-sum is computed as a chain of
one-hot matmuls on the PE:

    psum[block] += one_hot(ids_tile)^T @ feats_tile

Precision: feats are converted to a single bf16 copy on the host
(2 B/element — max rel err of the segment means ~2e-3, well under the
2e-2 gate), halving HBM traffic vs an fp32 or bf16-hi/lo scheme.  The
PE consumes bf16 at 1 cycle/row and accumulates fp32 in PSUM.

One-hots: ALL slots of a 16-tile chunk are built in ONE wide DVE
tensor_tensor is_equal, comparing a repeated iota (stride-0 broadcast
over the slot dim) against the per-slot rel values (stride-0 broadcast
over the 96 one-hot columns).  This amortizes the per-instruction DVE
overhead ~16x vs per-slot tensor_scalar ops.  (GpSimd is useless here:
its software tensor_scalar measures ~2.2us per 128x128 tile and
throttles the whole core to half clock.)

Block schedule: PSUM blocks cover 96 segments and start every 32
segments.  Each 128-row matmul tile is assigned, statically and
identically for all cores, to the one block that contains every core's
segment span for that tile (per-tile span ~50 segs incl. cross-core
skew, 96-32 = 64 of slack - fits).  No tile ever straddles a block
boundary, so there are zero duplicated matmuls.  The 64-segment
overlap between consecutive blocks is resolved by one constant-weight
shift matmul per block: rows [32:96) of block k (copied PSUM->SBUF in
bf16 on the otherwise idle Activation engine) are added into rows
[0:64) of block k+1 via an identity-like [96,64] weight.  Block k then
outputs rows [0:32) only (the last block outputs all 96).

DMA layout: rows are assigned to SBUF partitions chunk-wise
(partition p of a 2048-row chunk holds rows [16p, 16p+16)), which makes
every feats DMA a fully linear HBM read with 8 KiB contiguous packets
per partition.  The row permutation is absorbed into the precomputed
rel inputs.

SPMD: one Bass program runs on all 8 cores; all per-core differences
(row windows, relative ids, inverse counts) are carried in the input
data, never in the instruction stream.
"""

import numpy as np
import ml_dtypes

from concourse import bass, mybir, tile
from concourse.bass_utils import run_bass_kernel_spmd

N = 1_000_000
D = 256
S = 10_000
NCORES = 8
P = 128           # rows per matmul tile == SBUF partitions
CHUNK = 16        # tiles per feats DMA == consecutive rows per partition
SPC = S // NCORES # segments owned per core
BSTRIDE = 32      # segment stride between (overlapping) PSUM blocks
BWIDTH = 96       # segments covered per PSUM block
SHIFT = BWIDTH - BSTRIDE  # rows shifted into the next block (64)
NBLK = (SPC - BWIDTH + BSTRIDE - 1) // BSTRIDE + 1
NSL_MAX = CHUNK + 4  # one-hot pool slot capacity per chunk

_f32 = mybir.dt.float32
_bf16 = mybir.dt.bfloat16


def _plan(ids, n_rows, n_cores, segs_per_core, nblk, chunk):
    """Host-side plan: per-core row windows + static (tile, block) issue list.

    Row order is partition-major within each P*chunk-row chunk: tile
    (c, n) covers rows {chunk_start + chunk*p + n : p in 0..P-1}.
    Blocks overlap: block b covers local segments [32b, 32b+96).
    Every tile gets one slot per block needed so that EVERY core's rows
    are covered; each row is assigned (via rel) to the first covering
    slot of its tile.  Returns (starts, R, issue, rel, first_slot,
    last_slot).
    """
    g = np.arange(n_cores + 1, dtype=np.int64) * segs_per_core
    b_rows = np.searchsorted(ids, g, side="left")
    spans = b_rows[1:] - b_rows[:-1]
    R = int(np.ceil(spans.max() / (P * chunk)) * (P * chunk))
    assert R <= n_rows and R >= spans.max()
    starts = np.minimum(b_rows[:-1], n_rows - R)
    T = R // P
    nchunk = T // chunk

    vals = np.stack([ids[s:s + R] for s in starts]).astype(np.int64)
    vals -= g[:-1, None]
    vals_t = vals.reshape(n_cores, nchunk, P, chunk)
    owned = (vals_t >= 0) & (vals_t < segs_per_core)

    issue = []
    for c in range(nchunk):
        for n in range(chunk):
            v = vals_t[:, c, :, n]
            ok = owned[:, c, :, n]
            if not ok.any():
                continue
            lo = int(v[ok].min())
            hi = int(v[ok].max())
            b = min(lo // BSTRIDE, nblk - 1)
            bs = [b]
            while hi >= BSTRIDE * bs[-1] + BWIDTH:
                nb = min(bs[-1] + BWIDTH // BSTRIDE, hi // BSTRIDE, nblk - 1)
                assert nb > bs[-1], (lo, hi, bs)
                bs.append(nb)
            assert hi < BSTRIDE * bs[-1] + BWIDTH
            t = c * chunk + n
            issue.extend((t, b) for b in bs)
            assert len(bs) <= NSL_MAX - chunk + 1

    n_slots = len(issue)
    rel = np.full((n_cores, P, n_slots), -1.0, dtype=np.float32)
    assigned = np.zeros((n_cores, nchunk, P, chunk), dtype=bool)
    for i, (t, b) in enumerate(issue):
        v = vals_t[:, t // chunk, :, t % chunk]
        ok = owned[:, t // chunk, :, t % chunk]
        a = assigned[:, t // chunk, :, t % chunk]
        w = v - b * BSTRIDE
        hit = ok & ~a & (w >= 0) & (w < BWIDTH)
        rel[:, :, i] = np.where(hit, w, -1).astype(np.float32)
        a |= hit
    assert assigned[owned].all(), "some rows not covered by any slot"

    first_slot, last_slot = {}, {}
    for i, (t, b) in enumerate(issue):
        first_slot.setdefault(b, i)
        last_slot[b] = i
    assert set(first_slot) == set(range(nblk)), (
        f"blocks missing from issue list: {sorted(set(range(nblk)) - set(first_slot))}"
    )
    for b in range(1, nblk):
        # shift chain: block b stops at least 2 slots after b-1 (so the
        # shift-in lands after b's first start=True matmul and before
        # its stop), and b's accumulation starts by the slot right
        # after b-1's stop
        assert last_slot[b] >= last_slot[b - 1] + 2, (b, last_slot)
        assert first_slot[b] <= last_slot[b - 1] + 1, (b, first_slot, last_slot)
    return starts, R, issue, rel, first_slot, last_slot


def _build_program(R, d, nblk, issue, first_slot, last_slot, chunk):
    """Emit the SPMD Bass program (identical for all cores)."""
    T = R // P
    n_slots = len(issue)
    out_rows = BSTRIDE * (nblk - 1) + BWIDTH
    nc = bass.Bass()
    hb_d = nc.dram_tensor("hb", [R, d], _bf16, kind="ExternalInput")
    iota_d = nc.dram_tensor("iota", [P, P], _bf16, kind="ExternalInput")
    relb_d = nc.dram_tensor("relb", [P, n_slots], _bf16, kind="ExternalInput")
    # inv columns (f32) + the shift-weight scalar column (p - 32)
    meta_d = nc.dram_tensor("meta", [P, nblk + 1], _f32, kind="ExternalInput")
    out_d = nc.dram_tensor("out", [out_rows, d], _f32, kind="ExternalOutput")

    # slots per chunk (contiguous in issue order)
    chunk_slots = [[] for _ in range(T // chunk)]
    for i, (t, b) in enumerate(issue):
        chunk_slots[t // chunk].append(i)
    for sl in chunk_slots:
        assert sl == list(range(sl[0], sl[0] + len(sl))) if sl else True

    with tile.TileContext(nc) as tc:
        with (
            tc.tile_pool(name="const", bufs=1) as cpool,
            tc.tile_pool(name="feats", bufs=6) as fpool,
            tc.tile_pool(name="oh", bufs=3) as ohpool,
            tc.tile_pool(name="acc", bufs=5, space=bass.MemorySpace.PSUM) as pspool,
            tc.tile_pool(name="cpy", bufs=3) as cppool,
            tc.tile_pool(name="res", bufs=3) as rpool,
        ):
            iota_tile = cpool.tile([P, P], _bf16)
            nc.sync.dma_start(iota_tile[:], iota_d[:])
            relb_t = cpool.tile([P, n_slots], _bf16)
            nc.sync.dma_start(relb_t[:], relb_d[:])
            meta_t = cpool.tile([P, nblk + 1], _f32)
            nc.sync.dma_start(meta_t[:], meta_d[:])
            iota_t = iota_tile[:]
            inv_t = meta_t[:, 0:nblk]
            shsc_t = meta_t[:, nblk:]

            # shift weights: ones at (32+m, m), m in [0, 64)
            shift_w = cpool.tile([P, SHIFT], _bf16, name="shift_w")
            nc.vector.tensor_scalar(
                out=shift_w[:], in0=iota_t[:, 0:SHIFT],
                scalar1=shsc_t, scalar2=None,
                op0=mybir.AluOpType.is_equal)

            # PE warm-up: dummy matmuls while the first feats chunk is in
            # flight keep the PE busy so the clock ramps (0.65/1.2 ->
            # 2.4 GHz) before real work arrives.
            warm = cpool.tile([P, P], _bf16, name="warm")
            nc.vector.memset(warm[:], 0.0)
            warm_rhs = cpool.tile([P, d], _bf16, name="warm_rhs")
            nc.vector.memset(warm_rhs[:], 0.0)
            wacc = pspool.tile([P, d], _f32, name="wacc", tag="acc")
            for _ in range(24):
                nc.tensor.matmul(wacc[:], warm[:], warm_rhs[:],
                                 start=True, stop=True)

            psum_tiles = {}
            started = set()
            pending_shifts = []  # (emit_at_slot, b, pt)

            # Defer the PSUM->SBUF copy + shift matmul a few slots past
            # block b's stop so the PE doesn't stall on the Activation
            # copy at each block transition.
            SHIFT_DELAY = 6

            def emit_shift(b, pt):
                # add block b's rows [32:96) into block b+1's rows [0:64)
                # (constant-weight matmul via a bf16 SBUF copy made on the
                # otherwise idle Activation engine).  Copy only the live
                # [0:96) rows — [96:128) of the PSUM tile are never
                # written and could hold NaNs.
                cp = cppool.tile([BWIDTH, d], _bf16, name="cpy", tag="cpy")
                nc.scalar.activation(
                    cp[:], pt[0:BWIDTH, :], mybir.ActivationFunctionType.Copy)
                assert (b + 1) in psum_tiles and (b + 1) in started
                nc.tensor.matmul(psum_tiles[b + 1][0:SHIFT, :],
                                 shift_w[0:BWIDTH, :], cp[:],
                                 start=False, stop=False,
                                 skip_group_check=True)

            def emit_scale(b, pt):
                # block b fully accumulated: scale by 1/count, DMA out.
                rows = BWIDTH if b == nblk - 1 else BSTRIDE
                res = rpool.tile([rows, d], _f32, name="res", tag="res")
                nc.vector.tensor_scalar(
                    out=res[:], in0=pt[0:rows, :],
                    scalar1=inv_t[0:rows, b:b + 1], scalar2=None,
                    op0=mybir.AluOpType.mult)
                nc.scalar.dma_start(
                    out_d[b * BSTRIDE:b * BSTRIDE + rows, :], res[:])

            slot = 0
            for c in range(T // chunk):
                hl = fpool.tile([P, chunk, d], _bf16)
                r0 = c * chunk * P
                src = hb_d[r0:r0 + chunk * P].rearrange(
                    "(p n) d -> p n d", p=P)
                nc.sync.dma_start(hl[:], src)
                sl = chunk_slots[c]
                if not sl:
                    continue
                s0, nsl = sl[0], len(sl)
                assert s0 == slot and nsl <= NSL_MAX
                # all of this chunk's one-hots in one wide DVE op
                oh = ohpool.tile([P, NSL_MAX, BWIDTH], _bf16)
                nc.vector.tensor_tensor(
                    out=oh[:, 0:nsl, :],
                    in0=iota_t[:, None, 0:BWIDTH].broadcast_to([P, nsl, BWIDTH]),
                    in1=relb_t[:, s0:s0 + nsl, None].broadcast_to(
                        [P, nsl, BWIDTH]),
                    op=mybir.AluOpType.is_equal)
                for j in range(chunk):
                    t = c * chunk + j
                    while slot < n_slots and issue[slot][0] == t:
                        while pending_shifts and pending_shifts[0][0] <= slot:
                            _, pb, ppt = pending_shifts.pop(0)
                            emit_shift(pb, ppt)
                        b = issue[slot][1]
                        if b not in psum_tiles:
                            assert slot == first_slot[b], (slot, b)
                            psum_tiles[b] = pspool.tile(
                                [P, d], _f32, name="acc", tag="acc")
                            started.add(b)
                        pt = psum_tiles[b]
                        nc.tensor.matmul(pt[0:BWIDTH, :],
                                         oh[:, slot - s0, :], hl[:, j, :],
                                         start=(slot == first_slot[b]),
                                         stop=(slot == last_slot[b]),
                                         skip_group_check=True)
                        if slot == last_slot[b]:
                            pt = psum_tiles.pop(b)
                            if b + 1 < nblk:
                                # flush happens BEFORE a slot's matmul, so
                                # emit_at must exceed first_slot[b+1] (whose
                                # start=True matmul zeroes the PSUM block)
                                emit_at = min(slot + 1 + SHIFT_DELAY,
                                              last_slot[b + 1])
                                assert emit_at > first_slot[b + 1]
                                pending_shifts.append((emit_at, b, pt))
                            emit_scale(b, pt)
                        slot += 1
            for _, pb, ppt in pending_shifts:
                emit_shift(pb, ppt)
    assert slot == n_slots
    _strip_self_waits(nc)
    _legalize_waits(nc)
    return nc


# Compute ops whose ISA structs carry a single sync-wait slot.  Tile's
# pool-slot release join sometimes adds a same-engine WAW/WAR wait on top
# of a cross-engine one; same-engine ordering is already guaranteed by
# in-order execution (Tile records same-engine deps as no-sync edges
# elsewhere), so the self-wait is redundant and safe to drop.
_COMPUTE_OPS = (
    mybir.InstTensorTensor, mybir.InstTensorScalarPtr,
    mybir.InstTensorCopy, mybir.InstActivation, mybir.InstMemset,
    mybir.InstMatmult, mybir.InstLdweights, mybir.InstTensorReduce,
)

_COMPUTE_SEMS = ("PE_", "DVE_", "Pool_", "Activation_", "SP_")


def _strip_self_waits(nc):
    for bb in nc.main_func.blocks:
        for ins in bb.instructions:
            si = ins.sync_info
            if si is None or not si.on_wait:
                continue
            if isinstance(ins, _COMPUTE_OPS):
                eng = str(ins.engine).split(".")[-1]
                kept = [w for w in si.on_wait
                        if not w.ant_name.startswith(eng + "_")]
                if len(kept) != len(si.on_wait):
                    si.on_wait = kept
            elif isinstance(ins, mybir.InstDMACopy) and len(si.on_wait) > 1:
                # A WAW wait on the old writer's DMA queue is implied by the
                # compute-engine wait that gates on the old tile's readers
                # (the readers FIFO-follow a wait on that very queue).
                has_compute = any(
                    w.ant_name.startswith(_COMPUTE_SEMS) for w in si.on_wait)
                if has_compute:
                    kept = [w for w in si.on_wait
                            if not w.ant_name.startswith("DMAHW")]
                    if kept and len(kept) != len(si.on_wait):
                        si.on_wait = kept


def _legalize_waits(nc, maxw=1):
    """The walrus codegen here supports very few sync-wait commands per
    instruction.  Hoist excess waits onto preceding same-engine NoOps —
    engine FIFO order makes this equivalent."""
    for bb in nc.main_func.blocks:
        idx = 0
        while idx < len(bb.instructions):
            ins = bb.instructions[idx]
            si = ins.sync_info
            if si is not None and si.on_wait and len(si.on_wait) > maxw:
                waits = list(si.on_wait)
                si.on_wait = waits[-maxw:]
                for w in waits[:-maxw]:
                    nop = mybir.InstNoOp(
                        name=nc.get_next_instruction_name(),
                        engine=ins.engine,
                        sync_info=mybir.SyncInfo(on_wait=[w], on_update=[]),
                        bass_nofuse=True,
                    )
                    bb.instructions.insert(idx, nop)
                    idx += 1
            idx += 1


def _prepare_inputs(feats, ids, n_cores, segs_per_core, nblk, starts, R, rel):
    """Per-core input maps: bf16 feats + bf16 rel + f32 meta."""
    n, d = feats.shape
    counts = np.bincount(ids, minlength=n_cores * segs_per_core).astype(np.float32)
    inv = (1.0 / np.maximum(counts, 1.0)).astype(np.float32)
    inv_pad = np.zeros(n_cores * segs_per_core + nblk * BSTRIDE + P, np.float32)
    inv_pad[:inv.shape[0]] = inv

    hb = feats.astype(ml_dtypes.bfloat16)

    n_slots = rel.shape[2]
    iota = np.broadcast_to(np.arange(P, dtype=np.float32), (P, P))
    shsc = (np.arange(P, dtype=np.float32) - BSTRIDE).reshape(P, 1)
    in_maps = []
    for c in range(n_cores):
        g0 = c * segs_per_core
        inv_c = inv_pad[g0:g0 + nblk * BSTRIDE + P].copy()
        inv_c[segs_per_core:] = 0.0
        meta = np.empty((P, nblk + 1), np.float32)
        for b in range(nblk):
            meta[:, b] = inv_c[b * BSTRIDE:b * BSTRIDE + P]
        meta[:, nblk:] = shsc
        in_maps.append({
            "hb": hb[starts[c]:starts[c] + R],
            "iota": iota.astype(ml_dtypes.bfloat16),
            "relb": rel[c].astype(ml_dtypes.bfloat16),
            "meta": meta,
        })
    return in_maps


def _run(feats, ids, trace=False, trace_cores=None):
    n, d = feats.shape
    starts, R, issue, rel, first_slot, last_slot = _plan(
        ids, n, NCORES, SPC, NBLK, CHUNK)
    nc = _build_program(R, d, NBLK, issue, first_slot, last_slot, CHUNK)
    in_maps = _prepare_inputs(feats, ids, NCORES, SPC, NBLK,
                              starts, R, rel)
    res = run_bass_kernel_spmd(nc, in_maps, list(range(NCORES)),
                               trace=trace, trace_cores=trace_cores)
    out = np.concatenate(
        [res.results[c]["out"][:SPC] for c in range(NCORES)], axis=0)
    return out, res


def kernel(feats, segment_ids, num_segments):
    feats = np.ascontiguousarray(np.asarray(feats), dtype=np.float32)
    ids = np.asarray(segment_ids).astype(np.int64)
    s = int(num_segments)
    assert feats.shape == (N, D) and ids.shape == (N,) and s == S, (
        "kernel is specialized for feats [1e6, 256], 1e4 segments")
    out, _ = _run(feats, ids)
    return out
